# revision 1
# baseline (speedup 1.0000x reference)
"""NT-Xent loss kernel for Trainium2 (8 NeuronCores, SPMD).

Math (matches the reference exactly):
  z = concat(z1, z2)                      (N=8192, D=256)
  zhat = z / ||z||                        (row-normalized)
  sim = (zhat @ zhat.T) / T               (T=0.5)
  sim[diag] = -1e9
  loss = mean_i( lse_i - sim[i, label_i] )
       = ( sum_i lse_i + B*1e9 - sum_{i>=B} sim[i, i-B] ) / N
where lse_i = log(sum_j exp(sim_ij)) (the masked diag contributes
exp(-1e9+eps) == 0 in fp32, identical to the reference's behavior).

Sharding: rows of z across 8 cores (1024 rows each). Each core receives
the full z^T (D on partitions) with its columns rotated so that its own
rows sit at columns [0, 1024) — this makes the diagonal-mask and
positive-pair locations identical on every core (uniform SPMD program).

Per-core kernel (engines balanced so ScalarE's exp stream is the only
real floor: 8.4M exps/core at 1 elem/lane/cycle ~= 55us):
  - cast z^T to bf16 during the DMA load (SWDGE cast)
  - column norms^2 via ones-vector matmuls; inv = sqrt(2)/||z|| via DVE
    fast-rsqrt (int hack + 2 Newton steps) on a compact (w/64, 64)
    layout, broadcast across partitions with gpsimd.partition_broadcast,
    applied on DVE writing fp8e4
  - gram = znt_local^T @ znt in ONE fp8 DoubleRow matmul per 512-col
    chunk (both 128-deep K-tiles packed per PE cell, 0.5 cyc/row);
    the diagonal is masked by accumulating -1e9*I via an extra matmul
  - Exp with fused per-partition row-sum accumulation (accum_out) on
    ScalarE; only Exp/Ln from one activation-table set are used
  - positives extracted as colwise dot of znt[:, :, 0:1024] and
    znt[:, :, 4096:5120] via ones-vector matmuls
  - outputs [sum_local lse, sum_local pos] as (1, 2) f32

Column groups are processed in variable widths (1024, 1024, 2048, ...)
and each group's norm/inv chain is emitted two groups ahead of its
gram regions so the chain latency hides under the exp stream.

Host combines: loss = (sum lse + B*1e9 - sum_{cores 4..7} pos) / N.
All fp8/bf16 rounding lands in the lse/pos terms, whose contribution
to the loss is ~1e-4 absolute vs the exact B*1e9 term -- final relative
error vs the fp32 reference is ~5e-7.
"""

import math
from contextlib import ExitStack

import numpy as np

import concourse.bass as bass
import concourse.mybir as mybir
from concourse import bacc
from concourse.tile import TileContext
from concourse.bass_utils import run_bass_kernel_spmd

F32 = mybir.dt.float32
BF16 = mybir.dt.bfloat16
FP8 = mybir.dt.float8e4
AFT = mybir.ActivationFunctionType

B = 4096          # rows per view
D = 256           # feature dim
NTOT = 2 * B      # 8192 rows total
NCORES = 8
LOCAL = NTOT // NCORES   # 1024 rows per core
KT = D // 128            # 2 contraction tiles
NCH = 512                # matmul moving free dim
GW = 2048                # max column group width (4 chunks, 4 PSUM banks)
# variable-width column groups: narrow first groups shorten the critical
# chain to the first gram matmul (group 0 == the local/lhsT columns)
GROUPS = [(0, 1024), (1024, 1024), (2048, 2048), (4096, 2048), (6144, 2048)]
NG = len(GROUPS)
# gram/exp regions are uniform 2048-wide (fewer, bigger exp instructions);
# region b consumes chain groups covering its columns
RBLOCKS = [(0, 2048), (2048, 2048), (4096, 2048), (6144, 2048)]
NB = len(RBLOCKS)
MT = LOCAL // 128        # 8 row tiles per core
NEG = -1.0e9
HALF_LN2 = 0.5 * math.log(2.0)   # fold sqrt(1/T)=sqrt(2) into inv
SQRT2 = math.sqrt(2.0)


def build_nc():
    nc = bacc.Bacc("TRN2", target_bir_lowering=False, debug=False)
    zt = nc.dram_tensor("zt", [D, NTOT], F32, kind="ExternalInput")
    out = nc.dram_tensor("out", [1, 2], F32, kind="ExternalOutput")

    import ml_dtypes
    negeye_np = (np.eye(128, dtype=np.float32) * np.float32(NEG)).astype(ml_dtypes.bfloat16)
    negeye_d = nc.inline_tensor(negeye_np, name="negeye")
    eye_np = np.eye(128, dtype=np.float32).astype(ml_dtypes.bfloat16)
    eye_d = nc.inline_tensor(eye_np, name="eye_bf")

    with TileContext(nc) as tc, ExitStack() as ctx:
        consts = ctx.enter_context(tc.tile_pool(name="consts", bufs=1))
        big = ctx.enter_context(tc.tile_pool(name="big", bufs=1))
        sqp = ctx.enter_context(tc.tile_pool(name="sqp", bufs=4))
        scrp = ctx.enter_context(tc.tile_pool(name="scrp", bufs=2))
        smallp = ctx.enter_context(tc.tile_pool(name="smallp", bufs=2))

        negeye = consts.tile([128, 128], BF16)
        nc.sync.dma_start(out=negeye[:], in_=negeye_d[:, :])
        eye_bf = consts.tile([128, 128], BF16)
        nc.sync.dma_start(out=eye_bf[:], in_=eye_d[:, :])
        ones_bf = consts.tile([128, 1], BF16)
        nc.vector.memset(ones_bf[:], 1.0)
        ones_f32 = consts.tile([128, 1], F32)
        nc.vector.memset(ones_f32[:], 1.0)

        zbf = [[big.tile([128, GROUPS[g][1]], BF16, name=f"zbf_{k}_{g}", tag=f"zbf_{k}_{g}")
                for g in range(NG)] for k in range(KT)]
        znt = big.tile([128, KT, NTOT], FP8, name="znt", tag="znt")
        binv = big.tile([128, NTOT], BF16, name="binv", tag="binv")
        n2row = big.tile([1, NTOT], F32, name="n2row", tag="n2row")
        n2c = big.tile([32, 64 * NG], F32, name="n2c", tag="n2c")
        rsq_y = big.tile([32, 64 * NG], F32, name="rsq_y", tag="rsq_y")
        rsq_t = big.tile([32, 64 * NG], F32, name="rsq_t", tag="rsq_t")
        invc = big.tile([32, 64 * NG], BF16, name="invc", tag="invc")
        invrow = big.tile([1, NTOT], BF16, name="invrow", tag="invrow")
        accs = big.tile([128, MT * NB], F32)

        # single shared PSUM pool (tag "reg": 2 slots x 4 banks)
        psm = ctx.enter_context(tc.tile_pool(name="psm", bufs=2, space="PSUM"))

        def emit_chain(g):
            """Norms + inv + scale for column group g: produces znt[:, :, off:off+w].
            Issues the group's input loads here (not upfront) so a later
            group's 2MB load cannot queue ahead of an earlier group's
            latency-critical 4KB compact/broadcast transfers."""
            off, w = GROUPS[g]
            for k in range(KT):
                nc.gpsimd.dma_start(
                    out=zbf[k][g][:],
                    in_=zt[k * 128:(k + 1) * 128, off:off + w],
                )
            psA = psm.tile([128, GW], F32, name="reg", tag="reg")
            for k in range(KT):
                sq = sqp.tile([128, GW], BF16, name="sq", tag="sq")
                nc.vector.tensor_mul(sq[0:128, 0:w], zbf[k][g][:], zbf[k][g][:])
                for j in range(w // NCH):
                    nc.tensor.matmul(
                        psA[0:1, j * NCH:(j + 1) * NCH],
                        lhsT=ones_bf[:, 0:1],
                        rhs=sq[:, j * NCH:(j + 1) * NCH],
                        start=(k == 0),
                        stop=(k == KT - 1),
                    )
            if g <= 2:
                # prologue chains: drain on the idle ScalarE (Copy is in the
                # loaded table set; ScE has the faster PSUM port) so the DVE
                # can run sq/rsqrt/scale in parallel
                nc.scalar.copy(n2row[0:1, off:off + w], psA[0:1, 0:w])
            else:
                # mid-stream chains: keep the drain off ScalarE so it never
                # stalls the exp stream
                nc.vector.tensor_copy(n2row[0:1, off:off + w], psA[0:1, 0:w])
            # compact (1,w) -> (w//64, 64) in this group's column band
            nc.sync.dma_start(
                out=n2c[0:w // 64, 64 * g:64 * (g + 1)],
                in_=n2row[0:1, off:off + w],
            )
            # inv = sqrt(2)/||z|| via DVE fast-rsqrt (int hack + 2 Newton
            # steps, rel err ~1e-6) -- keeps the whole inv chain off ScalarE
            # so the activation table never leaves the exp set mid-kernel.
            gp = slice(0, w // 64)
            gcol = slice(64 * g, 64 * (g + 1))
            x = n2c[gp, gcol]
            y = rsq_y[gp, gcol]
            yi = rsq_y.bitcast(mybir.dt.int32)[gp, gcol]
            xi = n2c.bitcast(mybir.dt.int32)[gp, gcol]
            # y_int = 0x5f3759df - (x_int >> 1)
            nc.vector.tensor_scalar(
                out=yi, in0=xi, scalar1=1, scalar2=None,
                op0=mybir.AluOpType.arith_shift_right,
            )
            nc.vector.tensor_scalar(
                out=yi, in0=yi, scalar1=-1, scalar2=0x5F3759DF,
                op0=mybir.AluOpType.mult, op1=mybir.AluOpType.add,
            )
            for it in range(2):
                t = rsq_t[gp, gcol]
                nc.vector.tensor_mul(t, y, y)
                nc.vector.tensor_mul(t, t, x)
                nc.vector.tensor_scalar(
                    out=t, in0=t, scalar1=-0.5, scalar2=1.5,
                    op0=mybir.AluOpType.mult, op1=mybir.AluOpType.add,
                )
                if it < 1:
                    nc.vector.tensor_mul(y, y, t)
                else:
                    # fold the sqrt(2) temperature factor into the last step
                    nc.vector.scalar_tensor_tensor(
                        out=invc[gp, gcol], in0=y, scalar=SQRT2, in1=t,
                        op0=mybir.AluOpType.mult, op1=mybir.AluOpType.mult,
                    )
            nc.sync.dma_start(
                out=invrow[0:1, off:off + w],
                in_=invc[0:w // 64, 64 * g:64 * (g + 1)],
            )
            nc.gpsimd.partition_broadcast(
                out_ap=binv[:, off:off + w],
                in_ap=invrow[0:1, off:off + w],
            )
            for k in range(KT):
                nc.vector.tensor_mul(
                    znt[:, k, off:off + w],
                    zbf[k][g][:],
                    binv[:, off:off + w],
                )

        def emit_region(m, b):
            """Gram block (128 rows x w cols) + fused exp row-sums."""
            off, w = RBLOCKS[b]
            reg = psm.tile([128, GW], F32, name="reg", tag="reg")
            jdiag = (m * 128) // NCH if b == 0 else -1   # diag cols are in block 0
            for j in range(w // NCH):
                cc = off + j * NCH
                nc.tensor.matmul(
                    reg[:, j * NCH:(j + 1) * NCH],
                    lhsT=znt[:, :, m * 128:(m + 1) * 128],
                    rhs=znt[:, :, cc:cc + NCH],
                    start=True,
                    stop=(j != jdiag),
                    perf_mode=mybir.MatmulPerfMode.DoubleRow,
                )
                if j == jdiag:
                    # mask the self-similarity diagonal by accumulating
                    # -1e9 * I into its 128-col block
                    dcol = (m * 128) % NCH
                    nc.tensor.matmul(
                        reg[:, j * NCH + dcol:j * NCH + dcol + 128],
                        lhsT=negeye[:, :],
                        rhs=eye_bf[:, :],
                        start=False,
                        stop=True,
                    )
            scr = scrp.tile([128, GW], BF16, name="scr", tag="scr")
            idx = m * NB + b
            nc.scalar.activation(
                out=scr[0:128, 0:w],
                in_=reg[:, 0:w],
                func=AFT.Exp,
                accum_out=accs[:, idx:idx + 1],
            )

        def emit_pos():
            # positive-pair sums: colwise dot of znt[:, :, 0:1024] with
            # znt[:, :, 4096:5120]; runs mid-stream once group 2 is scaled
            pos_slot = psm.tile([128, GW], F32, name="reg", tag="reg")
            pos_ps = pos_slot[0:1, 0:LOCAL]
            for k in range(KT):
                prod = sqp.tile([128, LOCAL], BF16, name="prod", tag="prod")
                nc.vector.tensor_mul(
                    prod[:], znt[:, k, 0:LOCAL], znt[:, k, B:B + LOCAL]
                )
                for j in range(LOCAL // NCH):
                    nc.tensor.matmul(
                        pos_ps[0:1, j * NCH:(j + 1) * NCH],
                        lhsT=ones_bf[:, 0:1],
                        rhs=prod[:, j * NCH:(j + 1) * NCH],
                        start=(k == 0),
                        stop=(k == KT - 1),
                    )
            pos_tot = smallp.tile([1, 1], F32, name="pos_tot", tag="pos_tot")
            nc.vector.reduce_sum(out=pos_tot[:], in_=pos_ps[:], axis=mybir.AxisListType.X)
            return pos_tot

        # interleave: each region block's producer chains are emitted well
        # before its regions so norm/inv chains overlap the exp stream
        emit_chain(0)
        emit_chain(1)
        emit_chain(2)
        for b in range(NB):
            if b + 2 < NB:
                emit_chain(b + 3)    # chain g feeds block g-1 (g >= 2)
            for m in range(MT):
                emit_region(m, b)
        pos_tot = emit_pos()

        # ---- tail: lse, partition sums, output ----
        S = smallp.tile([128, MT], F32, name="S", tag="S")
        nc.vector.reduce_sum(
            out=S[:],
            in_=accs[:].rearrange("p (m b) -> p m b", b=NB),
            axis=mybir.AxisListType.X,
        )
        lse = smallp.tile([128, MT], F32, name="lse", tag="lse")
        nc.scalar.activation(out=lse[:], in_=S[:], func=AFT.Ln)
        lsesum = smallp.tile([128, 1], F32, name="lsesum", tag="lsesum")
        nc.vector.reduce_sum(out=lsesum[:], in_=lse[:], axis=mybir.AxisListType.X)

        tot_slot = psm.tile([128, GW], F32, name="reg", tag="reg")
        tot_ps = tot_slot[0:1, 0:1]
        nc.tensor.matmul(
            tot_ps, lhsT=lsesum[:, 0:1], rhs=ones_f32[:, 0:1],
            start=True, stop=True,
        )

        outsb = smallp.tile([1, 2], F32, name="outsb", tag="outsb")
        nc.vector.tensor_copy(outsb[0:1, 0:1], tot_ps)
        nc.vector.tensor_copy(outsb[0:1, 1:2], pos_tot[0:1, 0:1])
        nc.sync.dma_start(out=out[:, :], in_=outsb[:])

    # Bind both Exp and Ln to the one table set that contains them
    # (natural_log_exp_and_others) so the kernel performs a single
    # LoadActFuncSet instead of exp-set at start + ln-set on the tail.
    # Indices (= act_func_set_id) are preserved; guarded fallback.
    import concourse.bacc as _bacc_mod
    _orig_tables = _bacc_mod.get_activation_tables

    def _pinned_tables(arch):
        tabs = _orig_tables(arch)
        both = tabs.get("natural_log_exp_and_others")
        if not both or AFT.Exp not in both or AFT.Ln not in both:
            return tabs
        return {
            name: (fns if name == "natural_log_exp_and_others"
                   else fns - {AFT.Exp, AFT.Ln})
            for name, fns in tabs.items()
        }

    _bacc_mod.get_activation_tables = _pinned_tables
    try:
        nc.compile()
    finally:
        _bacc_mod.get_activation_tables = _orig_tables
    return nc


_NC_CACHE = None


def _get_nc():
    global _NC_CACHE
    if _NC_CACHE is None:
        _NC_CACHE = build_nc()
    return _NC_CACHE


def make_in_maps(z1: np.ndarray, z2: np.ndarray):
    z = np.concatenate([np.asarray(z1), np.asarray(z2)], axis=0)   # (8192, 256)
    zT = np.ascontiguousarray(z.T.astype(np.float32))              # (256, 8192)
    in_maps = []
    for c in range(NCORES):
        in_maps.append({"zt": np.ascontiguousarray(np.roll(zT, -c * LOCAL, axis=1))})
    return in_maps


def combine(parts):
    """parts: list of 8 (1,2) arrays [sum_lse, sum_pos] -> scalar loss (f32)."""
    sum_lse = sum(float(p[0, 0]) for p in parts)
    sum_pos = sum(float(p[0, 1]) for p in parts[NCORES // 2:])
    loss = (sum_lse + float(B) * 1.0e9 - sum_pos) / float(NTOT)
    return np.float32(loss)


def kernel(z1: np.ndarray, z2: np.ndarray) -> np.ndarray:
    nc = _get_nc()
    in_maps = make_in_maps(z1, z2)
    res = run_bass_kernel_spmd(nc, in_maps, core_ids=list(range(NCORES)))
    parts = [r["out"] for r in res.results]
    return combine(parts)



# revision 45
# speedup vs baseline: 1.4282x; 1.4282x over previous
"""NT-Xent loss kernel for Trainium2 (8 NeuronCores, SPMD, symmetric-pair
sharding).

Math (matches the reference exactly):
  z = concat(z1, z2)                      (N=8192, D=256)
  zhat = z / ||z||
  sim = (zhat @ zhat.T) / T               (T=0.5)
  sim[diag] = -1e9
  loss = mean_i( lse_i - sim[i, label_i] )
       = ( sum_i lse_i + B*1e9 - sum_{i>=B} sim[i, i-B] ) / N

Sharding exploits sim's SYMMETRY: exp(sim) is symmetric, so row sums of
the full matrix can be assembled from row sums of an upper-triangle-ish
block set plus COLUMN sums of the same blocks (colsum of block (r,s) ==
rowsum contribution for the rows of block s).  With rows in 8 blocks of
1024 (core c owns block c; all indexing below is in each core's ROTATED
frame where its own block sits at columns [0,1024)):

  core c computes blocks k = (s-c) mod 8:
    k=0      : full 1024 cols, rowsums only (diag block, self-symmetric)
    k=1,2,3  : full 1024 cols, rowsums + colsums (colsums shipped to
               core c+k, which owns those rows)
    k=4      : per 128-row tile u, only the 5 cell columns v-u mod 8 in
               {0..4} (640 cols).  Cells offset 1..3 also emit colsums
               (shipped to core c+4); offsets 0 and 4 are computed
               REDUNDANTLY by both cores of the pair (rowsums only) --
               +2.8% exp work buys a fully uniform SPMD program.

Per-core exp work: 8 * (4096 + 640) = 4.85M elems (vs 8.4M for the
row-parallel scheme).  ScalarE (the exp engine, 128 lanes @ 1.2GHz) is
the bottleneck; everything else hides under it:
  - norms via ones-matmuls in [1,512] chunks, fast-rsqrt (int hack + 2
    Newton steps) on DVE in compact (32,16)-per-chunk layout
  - gram in one fp8 DoubleRow matmul per 512-chunk (0.5 cyc/row)
  - exp on ScalarE with fused per-partition rowsum (accum_out), output
    fp8 to SBUF
  - colsums are ~FREE on the PE: matmul with lhsT = a 128-col chunk of
    the exp output and rhs = ones[128,1] gives that chunk's 128 column
    sums as out[128,1] (cost = 1 row); accumulated over the 8 row-tiles
    in a dedicated PSUM bank.
  - positives sim(r, 4096+r) via the same lhsT-colsum trick on an
    elementwise product of znt slabs.

Cross-core combine: each core outputs its partial rowsums [128,8], its
colsum partials [128,32] and positives [128,8] (one [128,48] f32 DMA).
The host PERMUTES these into per-core inputs for a tiny PHASE-2 device
program that sums the 5 contributions per row, takes Ln on-device,
reduces, and emits [sum_lse, sum_pos] per core.  Host combine is then 12
scalar adds:   loss = (sum lse + B*1e9 - sum_{c>=4} pos_c) / N.

All fp8/bf16 rounding lands in the lse/pos terms whose own relative
error stays ~1e-5; final loss rel err vs the f32 reference ~1e-7.
"""

import math
from contextlib import ExitStack

import numpy as np

import concourse.bass as bass
import concourse.mybir as mybir
from concourse import bacc
from concourse.tile import TileContext
from concourse.bass_utils import run_bass_kernel_spmd

F32 = mybir.dt.float32
BF16 = mybir.dt.bfloat16
FP8 = mybir.dt.float8e4
I32 = mybir.dt.int32
AFT = mybir.ActivationFunctionType

B = 4096          # rows per view
D = 256           # feature dim
NTOT = 2 * B      # 8192 rows total
NCORES = 8
LOCAL = NTOT // NCORES   # 1024 rows per core
KT = D // 128            # 2 contraction tiles
WCOLS = 5120             # rotated columns loaded per core
MT = 8                   # 128-row tiles per core
RW = 1536                # exp region width (3 PSUM banks)
NEG = -1.0e9
SQRT2 = math.sqrt(2.0)   # fold sqrt(1/T) into inv so gram == sim

# norm chains (col ranges); ch0 is tiny so the first region starts fast
CHAINS = [
    (0, 128),
    (128, 1536),
    (1536, 3072),
    (3072, 5120),
]
NQ = WCOLS // 128   # 40 128-col norm chunks; n2/inv live as [128, NQ]


def _rsqrt_chain(nc, x, y, yi, xi, t, out, newton=2):
    """inv = sqrt(2)/sqrt(x) via fast inverse sqrt + Newton steps.
    x,y,t f32 views; yi, xi int32 bitcasts of y, x; out bf16 view."""
    nc.vector.tensor_scalar(
        out=yi, in0=xi, scalar1=1, scalar2=None,
        op0=mybir.AluOpType.arith_shift_right,
    )
    nc.vector.tensor_scalar(
        out=yi, in0=yi, scalar1=-1, scalar2=0x5F3759DF,
        op0=mybir.AluOpType.mult, op1=mybir.AluOpType.add,
    )
    for it in range(newton):
        nc.vector.tensor_mul(t, y, y)
        nc.vector.tensor_mul(t, t, x)
        nc.vector.tensor_scalar(
            out=t, in0=t, scalar1=-0.5, scalar2=1.5,
            op0=mybir.AluOpType.mult, op1=mybir.AluOpType.add,
        )
        if it < newton - 1:
            nc.vector.tensor_mul(y, y, t)
        else:
            nc.vector.scalar_tensor_tensor(
                out=out, in0=y, scalar=SQRT2, in1=t,
                op0=mybir.AluOpType.mult, op1=mybir.AluOpType.mult,
            )


def build_nc1():
    """Phase 1: gram + exp + partial row/col sums."""
    nc = bacc.Bacc("TRN2", target_bir_lowering=False, debug=False)
    zt = nc.dram_tensor("zt", [D, WCOLS], BF16, kind="ExternalInput")
    out1 = nc.dram_tensor("out1", [128, 48], F32, kind="ExternalOutput")

    import ml_dtypes
    negeye_np = (np.eye(128, dtype=np.float32) * np.float32(NEG)).astype(ml_dtypes.bfloat16)
    negeye_d = nc.inline_tensor(negeye_np, name="negeye")
    eye_np = np.eye(128, dtype=np.float32).astype(ml_dtypes.bfloat16)
    eye_d = nc.inline_tensor(eye_np, name="eye_bf")

    with TileContext(nc) as tc, ExitStack() as ctx:
        consts = ctx.enter_context(tc.tile_pool(name="consts", bufs=1))
        big = ctx.enter_context(tc.tile_pool(name="big", bufs=1))
        scrp = ctx.enter_context(tc.tile_pool(name="scrp", bufs=4))
        scrpp = ctx.enter_context(tc.tile_pool(name="scrpp", bufs=2))
        smallp = ctx.enter_context(tc.tile_pool(name="smallp", bufs=2))

        # ---- PSUM: 2x[128,1536] exp slots + [128,512] psC + [128,512] spare
        psm = ctx.enter_context(tc.tile_pool(name="psm", bufs=2, space="PSUM"))
        pscp = ctx.enter_context(tc.tile_pool(name="pscp", bufs=1, space="PSUM"))
        pspare = ctx.enter_context(tc.tile_pool(name="pspare", bufs=1, space="PSUM"))

        # ---- constants / big tensors
        dummy_in = consts.tile([1, 1], F32, name="dummy_in", tag="dummy_in")
        nc.vector.memset(dummy_in[:], 0.0)
        dummy_out = consts.tile([1, 1], F32, name="dummy_out", tag="dummy_out")
        # early Exp: pulls the activation-table load off the critical path
        nc.scalar.activation(out=dummy_out[:], in_=dummy_in[:], func=AFT.Exp)

        zbf = big.tile([128, KT, WCOLS], BF16, name="zbf", tag="zbf")
        znt = big.tile([128, KT, WCOLS], FP8, name="znt", tag="znt")
        invrow = big.tile([1, WCOLS], BF16, name="invrow", tag="invrow")
        n2r0 = big.tile([1, 128], F32, name="n2r0", tag="n2r0")
        d_y = big.tile([1, 128], F32, name="d_y", tag="d_y")
        d_t = big.tile([1, 128], F32, name="d_t", tag="d_t")
        n2sb = big.tile([128, NQ], F32, name="n2sb", tag="n2sb")
        rsq_y = big.tile([128, NQ], F32, name="rsq_y", tag="rsq_y")
        rsq_t = big.tile([128, NQ], F32, name="rsq_t", tag="rsq_t")
        accs = big.tile([128, MT, 5], F32, name="accs", tag="accs")
        nc.vector.memset(accs[:], 0.0)

        negeye = consts.tile([128, 128], BF16, name="negeye", tag="negeye")
        eye_bf = consts.tile([128, 128], BF16, name="eye_bf", tag="eye_bf")
        ones_bf = consts.tile([128, 1], BF16, name="ones_bf", tag="ones_bf")
        nc.vector.memset(ones_bf[:], 1.0)
        ones_f8 = consts.tile([128, 1], FP8, name="ones_f8", tag="ones_f8")
        nc.vector.memset(ones_f8[:], 1.0)
        ones_dr = consts.tile([128, KT, 1], FP8, name="ones_dr", tag="ones_dr")
        nc.vector.memset(ones_dr[:], 1.0)
        ones1 = consts.tile([1, 128], BF16, name="ones1", tag="ones1")
        nc.vector.memset(ones1[:], 1.0)

        # psC: colsum/pos accumulators, one PSUM bank.
        # cols 0..23  : full-block colsums, block k chunk m -> col (k-1)*8+m
        # cols 24..31 : block-4 cell colsums, col-tile v -> col 24+v
        # cols 32..39 : positives (8 chunks of 128)
        # cols 64..103: chain norms;  256..383 (f32): ch0 norm row
        #
        # PSUM start=True lazily zero-arms the whole 2KB bank row, so a bank
        # with many long-lived accumulators must see exactly ONE start (the
        # opener below) and ONE stop (the closer at the end); every other
        # matmul into it uses start=False (first touch of an armed byte
        # writes, later touches accumulate).
        psC = pscp.tile([128, 512], F32, name="psC", tag="psC")
        nc.tensor.matmul(psC[:, 448:449], lhsT=ones1[0:1, :],
                         rhs=ones1[0:1, 0:1], start=True, stop=False)

        def emit_load(ci):
            # all loads on one queue: strict priority order on the shared
            # HWDGE/DMA devices (ch0 first, then consts, then the rest);
            # both k-planes in ONE DMA (3-dim DRAM AP) to halve HWDGE holds
            lo, hi = CHAINS[ci]
            nc.sync.dma_start(
                out=zbf[:, :, lo:hi],
                in_=zt[:, lo:hi].rearrange("(k p) c -> p k c", p=128),
            )
            if ci == 0:
                nc.sync.dma_start(out=negeye[:], in_=negeye_d[:, :])
                nc.sync.dma_start(out=eye_bf[:], in_=eye_d[:, :])

        sq_tiles = {}

        def emit_sq(ci):
            """One Square activation per chain on the (idle-during-boot)
            ScalarE; Square is in the same table set as Exp (no reload)."""
            lo, hi = CHAINS[ci]
            sqt = big.tile([128, KT, hi - lo], BF16, name=f"sq{ci}", tag=f"sq{ci}")
            nc.scalar.square(sqt[:, :, :], zbf[:, :, lo:hi])
            sq_tiles[ci] = sqt

        def emit_norms(ci):
            """Norms via the colsum trick, in PARTITION-OUTER compact
            layout: matmul #q uses lhsT = the stride-nq column comb
            sq[:, k, q::nq], so psum col q holds n2(lo + p*nq + q) on
            partition p.  The uncompact DMA is then the plain
            partition-outer [128,nq] -> [1,w] pattern (baseline-proven)."""
            lo, hi = CHAINS[ci]
            q0, q1 = lo // 128, hi // 128
            nq = q1 - q0
            sqt = sq_tiles[ci]
            sqv = sqt[:, :, :].rearrange("p k (c q) -> p k q c", q=nq)
            for q in range(nq):
                for k in range(KT):
                    nc.tensor.matmul(
                        psC[:, 64 + q0 + q:65 + q0 + q],
                        lhsT=sqv[:, k, q, :],
                        rhs=ones_bf[:, 0:1],
                        start=False, stop=False,
                    )
            nc.vector.tensor_copy(n2sb[:, q0:q1], psC[:, 64 + q0:64 + q1])
            x = n2sb[:, q0:q1]
            xi = n2sb.bitcast(I32)[:, q0:q1]
            y = rsq_y[:, q0:q1]
            yi = rsq_y.bitcast(I32)[:, q0:q1]
            t_ = rsq_t[:, q0:q1]
            invcc = big.tile([128, nq], BF16, name=f"invcc{ci}",
                             tag=f"invcc{ci}")
            _rsqrt_chain(nc, x, y, yi, xi, t_, invcc[:, :])
            # uncompact: invcc[p, q] = inv of col lo + p*nq + q -> invrow
            nc.sync.dma_start(out=invrow[0:1, lo:hi], in_=invcc[:, :])

        def emit_scale(c0, c1, pso_pool, pso_w):
            """znt[:, :, c0:c1] = zbf * inv_col, where inv_col comes from a
            rank-1 PE outer product ones[1,128]^T @ invrow-slice held in
            PSUM and read directly by the DVE scale (no binv tensor)."""
            for g0 in range(c0, c1, pso_w):
                g1 = min(g0 + pso_w, c1)
                pso = pso_pool.tile([128, pso_w], F32, name="pso",
                                    tag="reg" if pso_pool is psm else "spare")
                for s0 in range(g0, g1, 512):
                    s1 = min(s0 + 512, g1)
                    nc.tensor.matmul(
                        pso[:, s0 - g0:s1 - g0], lhsT=ones1[0:1, :],
                        rhs=invrow[0:1, s0:s1], start=True, stop=True,
                    )
                for k in range(KT):
                    nc.vector.tensor_mul(
                        znt[:, k, g0:g1], zbf[:, k, g0:g1], pso[:, 0:g1 - g0])

        def emit_chain0():
            """Cols [0,128): minimal-latency direct path (no DMA hops):
            ones-matmul n2 row + DVE drain + 1-Newton rsqrt on [1,128]
            (rel err ~1e-3; these 128 cols are 2.7% of each row sum)."""
            lo, hi = CHAINS[0]
            emit_sq(0)
            sqt = sq_tiles[0]
            for k in range(KT):
                nc.tensor.matmul(
                    psC[0:1, 256:384], lhsT=ones_bf[:, 0:1], rhs=sqt[:, k, 0:128],
                    start=False, stop=False,
                )
            nc.vector.tensor_copy(n2r0[0:1, :], psC[0:1, 256:384])
            x = n2r0[0:1, :]
            xi = n2r0.bitcast(I32)[0:1, :]
            y = d_y[0:1, :]
            yi = d_y.bitcast(I32)[0:1, :]
            _rsqrt_chain(nc, x, y, yi, xi, d_t[0:1, :], invrow[0:1, lo:hi],
                         newton=1)
            emit_scale(lo, hi, pspare, 512)

        # region list: (m, reg_off, reg_w, acc_t);  m0/r0 split for startup
        regions = []
        for r in range(3):
            for m in range(MT):
                if r == 0 and m == 0:
                    regions.append((0, 0, 128, 0))
                    regions.append((0, 128, 1408, 1))
                else:
                    regions.append((m, 1536 * r, 1536, 2 + r if m == 0 else r))

        # block-4 cell colsum start/stop bookkeeping: psC col 24+v gets
        # contributions from m in sorted({v-1, v-2, v-3} mod 8)
        cell_ms = {v: sorted(((v - k) % 8 for k in range(1, 4))) for v in range(8)}

        pending_colsums = []  # deferred one region for PE pipelining

        def flush_colsums():
            for args in pending_colsums:
                nc.tensor.matmul(**args)
            pending_colsums.clear()

        def emit_region(m, off, w, acc_t):
            reg = psm.tile([128, RW], F32, name="reg", tag="reg")
            # gram chunks (512-aligned within the tile)
            for j in range((w + 511) // 512):
                l0, l1 = 512 * j, min(512 * (j + 1), w)
                g0 = off + l0
                if off == 3072 and l0 == 1024:
                    # r2 cells: 4 x [128,128] at col-tiles (m+k)%8 of block 4
                    for kk in range(4):
                        v = (m + kk) % 8
                        nc.tensor.matmul(
                            reg[:, l0 + 128 * kk:l0 + 128 * (kk + 1)],
                            lhsT=znt[:, :, 128 * m:128 * (m + 1)],
                            rhs=znt[:, :, 4096 + 128 * v:4096 + 128 * (v + 1)],
                            start=True, stop=True,
                            perf_mode=mybir.MatmulPerfMode.DoubleRow,
                        )
                    continue
                has_diag = g0 <= 128 * m and 128 * (m + 1) <= off + l1
                nc.tensor.matmul(
                    reg[:, l0:l1],
                    lhsT=znt[:, :, 128 * m:128 * (m + 1)],
                    rhs=znt[:, :, g0:off + l1],
                    start=True, stop=not has_diag,
                    perf_mode=mybir.MatmulPerfMode.DoubleRow,
                )
                if has_diag:
                    dcol = 128 * m - g0
                    nc.tensor.matmul(
                        reg[:, l0 + dcol:l0 + dcol + 128],
                        lhsT=negeye[:, :], rhs=eye_bf[:, :],
                        start=False, stop=True,
                    )
            flush_colsums()
            scr = scrp.tile([128, RW], FP8, name="scr", tag="scr")
            nc.scalar.activation(
                out=scr[0:128, 0:w], in_=reg[:, 0:w], func=AFT.Exp,
                accum_out=accs[:, m, acc_t:acc_t + 1],
            )
            # colsums for off-diag full-block cols [1024,4096) + r2 cells
            for gc in range(max(off, 1024), min(off + w, 4096), 128):
                lc = gc - off
                pc = (gc - 1024) // 128
                pending_colsums.append(dict(
                    out=psC[:, pc:pc + 1],
                    lhsT=scr[:, lc:lc + 128], rhs=ones_f8[:, 0:1],
                    start=False, stop=False,
                ))
            if off == 3072:
                for kk in range(1, 4):   # cells k=1..3 emit colsums
                    v = (m + kk) % 8
                    lc = 1024 + 128 * kk
                    pc = 24 + v
                    pending_colsums.append(dict(
                        out=psC[:, pc:pc + 1],
                        lhsT=scr[:, lc:lc + 128], rhs=ones_f8[:, 0:1],
                        start=False, stop=False,
                    ))

        # positives: sim(r, 4096+r) via elementwise prod + colsum trick
        prod = big.tile([128, KT, LOCAL], FP8, name="prod", tag="prod")

        def emit_pos_prods():
            for k in range(KT):
                nc.vector.tensor_mul(
                    prod[:, k, :], znt[:, k, 0:LOCAL], znt[:, k, 4096:4096 + LOCAL])

        def emit_pos_mms():
            for t in range(8):
                nc.tensor.matmul(
                    psC[:, 32 + t:33 + t],
                    lhsT=prod[:, :, 128 * t:128 * (t + 1)], rhs=ones_dr[:, :, 0:1],
                    start=False, stop=False,
                    perf_mode=mybir.MatmulPerfMode.DoubleRow,
                )

        # P regions: redundant offset-4 cells (u, u+4), rowsums via
        # segmented reduce (no accum: partitions span 4 different m-tiles)
        p4 = smallp.tile([128, 8], F32, name="p4", tag="p4")

        def emit_p(half):
            preg = pspare.tile([128, 512], F32, name="preg", tag="spare")
            for t in range(4):
                u = 4 * half + t
                v = (u + 4) % 8
                nc.tensor.matmul(
                    preg[:, 128 * t:128 * (t + 1)],
                    lhsT=znt[:, :, 128 * u:128 * (u + 1)],
                    rhs=znt[:, :, 4096 + 128 * v:4096 + 128 * (v + 1)],
                    start=(t == 0), stop=(t == 3),
                    perf_mode=mybir.MatmulPerfMode.DoubleRow,
                )
            scrP = scrpp.tile([128, 512], FP8, name="scrP", tag="scrP")
            nc.scalar.activation(out=scrP[:, 0:512], in_=preg[:, 0:512], func=AFT.Exp)
            nc.vector.reduce_sum(
                out=p4[:, 4 * half:4 * half + 4],
                in_=scrP[:].rearrange("p (c w) -> p c w", w=128),
                axis=mybir.AxisListType.X,
            )

        # ---- emission schedule: chains' latency pieces front-loaded so the
        # exp stream (ScalarE) never starves; squares use ScalarE's idle boot
        # window; pos/P interleave into the stream instead of after it.
        emit_load(0)
        emit_load(1)
        emit_load(2)
        emit_load(3)
        emit_chain0()
        emit_sq(1)
        emit_region(*regions[0])        # (m0, [0,128))
        emit_norms(1)
        emit_sq(2)
        emit_sq(3)
        emit_scale(128, 1536, psm, RW)  # via a borrowed exp slot (1 group)
        emit_region(*regions[1])        # (m0, [128,1536))
        emit_norms(2)
        emit_scale(1536, 3072, pspare, 512)
        emit_region(*regions[2])        # r0 m1
        emit_region(*regions[3])        # r0 m2
        emit_norms(3)
        for i in range(4, 9):           # r0 m3..m7
            emit_region(*regions[i])
        emit_scale(3072, 5120, pspare, 512)
        for i in range(9, 15):          # r1 m0..m5
            emit_region(*regions[i])
        emit_pos_prods()
        for i in range(15, 19):         # r1 m6..m7, r2 m0..m1
            emit_region(*regions[i])
        emit_pos_mms()
        for i in range(19, 22):         # r2 m2..m4
            emit_region(*regions[i])
        emit_p(0)
        for i in range(22, 24):         # r2 m5..m6
            emit_region(*regions[i])
        emit_p(1)
        emit_region(*regions[24])       # r2 m7
        flush_colsums()
        # close the psC bank's long accumulation group (sim bookkeeping)
        nc.tensor.matmul(psC[:, 449:450], lhsT=ones1[0:1, :],
                         rhs=ones1[0:1, 0:1], start=False, stop=True)

        # ---- tail: colsum/pos columns finalize ~2 exps before the stream
        # ends, so their DMA overlaps the last exps; only the tiny rowsum
        # DMA trails the final accum.
        sbout = big.tile([128, 48], F32, name="sbout", tag="sbout")
        nc.vector.tensor_copy(sbout[:, 8:48], psC[:, 0:40])
        nc.sync.dma_start(out=out1[:, 8:48], in_=sbout[:, 8:48])
        srow = smallp.tile([128, 8], F32, name="srow", tag="srow")
        nc.vector.reduce_sum(out=srow[:], in_=accs[:], axis=mybir.AxisListType.X)
        nc.vector.tensor_add(sbout[:, 0:8], srow[:], p4[:])
        nc.sync.dma_start(out=out1[:, 0:8], in_=sbout[:, 0:8])

    nc.compile()
    return nc


def build_nc2():
    """Phase 2: S = sum of 5 routed contributions per row; lse = ln S;
    emit per-core [sum_lse, sum_pos]."""
    nc = bacc.Bacc("TRN2", target_bir_lowering=False, debug=False)
    in2 = nc.dram_tensor("in2", [128, 48], F32, kind="ExternalInput")
    out2 = nc.dram_tensor("out2", [2, 1], F32, kind="ExternalOutput")

    with TileContext(nc) as tc, ExitStack() as ctx:
        p = ctx.enter_context(tc.tile_pool(name="p", bufs=1))
        ps = ctx.enter_context(tc.tile_pool(name="ps", bufs=1, space="PSUM"))

        dummy_in = p.tile([1, 1], F32, name="dummy_in", tag="dummy_in")
        nc.vector.memset(dummy_in[:], 1.0)
        dummy_out = p.tile([1, 1], F32, name="dummy_out", tag="dummy_out")
        nc.scalar.activation(out=dummy_out[:], in_=dummy_in[:], func=AFT.Ln)

        vin = p.tile([128, 48], F32, name="vin", tag="vin")
        nc.sync.dma_start(out=vin[:], in_=in2[:, :])

        S = p.tile([128, 8], F32, name="S", tag="S")
        nc.vector.reduce_sum(
            out=S[:],
            in_=vin[:, 0:40].rearrange("p (m j) -> p m j", j=5),
            axis=mybir.AxisListType.X,
        )
        lse = p.tile([128, 8], F32, name="lse", tag="lse")
        nc.scalar.activation(out=lse[:], in_=S[:], func=AFT.Ln)

        pk = p.tile([128, 2], F32, name="pk", tag="pk")
        nc.vector.reduce_sum(out=pk[:, 0:1], in_=lse[:], axis=mybir.AxisListType.X)
        nc.vector.reduce_sum(out=pk[:, 1:2], in_=vin[:, 40:48], axis=mybir.AxisListType.X)

        ones_f32 = p.tile([128, 1], F32, name="ones_f32", tag="ones_f32")
        nc.vector.memset(ones_f32[:], 1.0)
        tot = ps.tile([128, 512], F32, name="tot", tag="tot")
        nc.tensor.matmul(tot[0:2, 0:1], lhsT=pk[:, 0:2], rhs=ones_f32[:, 0:1],
                         start=True, stop=True)
        osb = p.tile([2, 1], F32, name="osb", tag="osb")
        nc.vector.tensor_copy(osb[:], tot[0:2, 0:1])
        nc.sync.dma_start(out=out2[:, :], in_=osb[:])

    nc.compile()
    return nc


_NC_CACHE = [None, None]


def _get_ncs():
    if _NC_CACHE[0] is None:
        _NC_CACHE[0] = build_nc1()
        _NC_CACHE[1] = build_nc2()
    return _NC_CACHE


def make_in_maps(z1: np.ndarray, z2: np.ndarray):
    import ml_dtypes
    z = np.concatenate([np.asarray(z1, np.float32), np.asarray(z2, np.float32)], axis=0)
    zT = np.ascontiguousarray(z.T)                      # (256, 8192) f32
    zTb = zT.astype(ml_dtypes.bfloat16)
    in_maps = []
    for c in range(NCORES):
        rolled = np.roll(zTb, -c * LOCAL, axis=1)[:, :WCOLS]
        in_maps.append({"zt": np.ascontiguousarray(rolled)})
    return in_maps


def route_phase2(outs1):
    """outs1: list of 8 [128,48] f32 arrays -> list of 8 [128,48] phase-2
    inputs.  Pure permutation/gather of columns (no arithmetic).

    out1 cols: 0..7  partial rowsums (m);  8+q colsum psC col q
               (q=(k-1)*8+m for full blocks k=1..3; q=24+v for block-4
               cells; q=32+t positives);  col j of in2: 5m+j layout."""
    in2 = []
    for s in range(NCORES):
        arr = np.zeros((128, 48), np.float32)
        own = outs1[s]
        for m in range(8):
            arr[:, 5 * m + 0] = own[:, m]
            for j in (1, 2, 3):
                peer = outs1[(s - j) % NCORES]
                arr[:, 5 * m + j] = peer[:, 8 + (j - 1) * 8 + m]
            peer4 = outs1[(s + 4) % NCORES]
            arr[:, 5 * m + 4] = peer4[:, 8 + 24 + m]
        arr[:, 40:48] = own[:, 40:48]
        in2.append({"in2": arr})
    return in2


def combine(parts2):
    """parts2: 8 x [2,1] arrays [sum_lse; sum_pos] -> scalar loss."""
    sum_lse = sum(float(p[0, 0]) for p in parts2)
    sum_pos = sum(float(p[1, 0]) for p in parts2[NCORES // 2:])
    loss = (sum_lse + float(B) * 1.0e9 - sum_pos) / float(NTOT)
    return np.float32(loss)


def kernel(z1: np.ndarray, z2: np.ndarray) -> np.ndarray:
    nc1, nc2 = _get_ncs()
    in_maps = make_in_maps(z1, z2)
    res1 = run_bass_kernel_spmd(nc1, in_maps, core_ids=list(range(NCORES)))
    outs1 = [np.asarray(r["out1"], np.float32) for r in res1.results]
    in2 = route_phase2(outs1)
    res2 = run_bass_kernel_spmd(nc2, in2, core_ids=list(range(NCORES)))
    parts2 = [np.asarray(r["out2"], np.float32) for r in res2.results]
    return combine(parts2)


# revision 54
# speedup vs baseline: 1.4604x; 1.0225x over previous
"""NT-Xent loss kernel for Trainium2 (8 NeuronCores, SPMD, symmetric-pair
sharding).

Math (matches the reference exactly):
  z = concat(z1, z2)                      (N=8192, D=256)
  zhat = z / ||z||
  sim = (zhat @ zhat.T) / T               (T=0.5)
  sim[diag] = -1e9
  loss = mean_i( lse_i - sim[i, label_i] )
       = ( sum_i lse_i + B*1e9 - sum_{i>=B} sim[i, i-B] ) / N

Sharding exploits sim's SYMMETRY: exp(sim) is symmetric, so row sums of
the full matrix can be assembled from row sums of an upper-triangle-ish
block set plus COLUMN sums of the same blocks (colsum of block (r,s) ==
rowsum contribution for the rows of block s).  With rows in 8 blocks of
1024 (core c owns block c; all indexing below is in each core's ROTATED
frame where its own block sits at columns [0,1024)):

  core c computes blocks k = (s-c) mod 8:
    k=0      : full 1024 cols, rowsums only (diag block, self-symmetric)
    k=1,2,3  : full 1024 cols, rowsums + colsums (colsums shipped to
               core c+k, which owns those rows)
    k=4      : per 128-row tile u, only the 5 cell columns v-u mod 8 in
               {0..4} (640 cols).  Cells offset 1..3 also emit colsums
               (shipped to core c+4); offsets 0 and 4 are computed
               REDUNDANTLY by both cores of the pair (rowsums only) --
               +2.8% exp work buys a fully uniform SPMD program.

Per-core exp work: 8 * (4096 + 640) = 4.85M elems (vs 8.4M for the
row-parallel scheme).  ScalarE (the exp engine, 128 lanes @ 1.2GHz) is
the bottleneck; everything else hides under it:
  - norms via ones-matmuls in [1,512] chunks, fast-rsqrt (int hack + 2
    Newton steps) on DVE in compact (32,16)-per-chunk layout
  - gram in one fp8 DoubleRow matmul per 512-chunk (0.5 cyc/row)
  - exp on ScalarE with fused per-partition rowsum (accum_out), output
    fp8 to SBUF
  - colsums are ~FREE on the PE: matmul with lhsT = a 128-col chunk of
    the exp output and rhs = ones[128,1] gives that chunk's 128 column
    sums as out[128,1] (cost = 1 row); accumulated over the 8 row-tiles
    in a dedicated PSUM bank.
  - positives sim(r, 4096+r) via the same lhsT-colsum trick on an
    elementwise product of znt slabs.

Cross-core combine: each core outputs its partial rowsums [128,8], its
colsum partials [128,32] and positives [128,8] (one [128,48] f32 DMA).
The host PERMUTES these into per-core inputs for a tiny PHASE-2 device
program that sums the 5 contributions per row, takes Ln on-device,
reduces, and emits [sum_lse, sum_pos] per core.  Host combine is then 12
scalar adds:   loss = (sum lse + B*1e9 - sum_{c>=4} pos_c) / N.

All fp8/bf16 rounding lands in the lse/pos terms whose own relative
error stays ~1e-5; final loss rel err vs the f32 reference ~1e-7.
"""

import math
from contextlib import ExitStack

import numpy as np

import concourse.bass as bass
import concourse.mybir as mybir
from concourse import bacc
from concourse.tile import TileContext
from concourse.bass_utils import run_bass_kernel_spmd

F32 = mybir.dt.float32
BF16 = mybir.dt.bfloat16
FP8 = mybir.dt.float8e4
I32 = mybir.dt.int32
AFT = mybir.ActivationFunctionType

B = 4096          # rows per view
D = 256           # feature dim
NTOT = 2 * B      # 8192 rows total
NCORES = 8
LOCAL = NTOT // NCORES   # 1024 rows per core
KT = D // 128            # 2 contraction tiles
WCOLS = 5120             # rotated columns loaded per core
MT = 8                   # 128-row tiles per core
RW = 1536                # exp region width (3 PSUM banks)
NEG = -1.0e9
SQRT2 = math.sqrt(2.0)   # fold sqrt(1/T) into inv so gram == sim

# norm chains (col ranges); ch0 is tiny so the first region starts fast
CHAINS = [
    (0, 128),
    (128, 1536),
    (1536, 3072),
    (3072, 5120),
]
NQ = WCOLS // 128   # 40 128-col norm chunks; n2/inv live as [128, NQ]


def _rsqrt_chain(nc, x, y, yi, xi, t, out, newton=2):
    """inv = sqrt(2)/sqrt(x) via fast inverse sqrt + Newton steps.
    x,y,t f32 views; yi, xi int32 bitcasts of y, x; out bf16 view."""
    nc.vector.tensor_scalar(
        out=yi, in0=xi, scalar1=1, scalar2=None,
        op0=mybir.AluOpType.arith_shift_right,
    )
    nc.vector.tensor_scalar(
        out=yi, in0=yi, scalar1=-1, scalar2=0x5F3759DF,
        op0=mybir.AluOpType.mult, op1=mybir.AluOpType.add,
    )
    for it in range(newton):
        nc.vector.tensor_mul(t, y, y)
        nc.vector.tensor_mul(t, t, x)
        nc.vector.tensor_scalar(
            out=t, in0=t, scalar1=-0.5, scalar2=1.5,
            op0=mybir.AluOpType.mult, op1=mybir.AluOpType.add,
        )
        if it < newton - 1:
            nc.vector.tensor_mul(y, y, t)
        else:
            nc.vector.scalar_tensor_tensor(
                out=out, in0=y, scalar=SQRT2, in1=t,
                op0=mybir.AluOpType.mult, op1=mybir.AluOpType.mult,
            )


def build_nc1():
    """Phase 1: gram + exp + partial row/col sums."""
    nc = bacc.Bacc("TRN2", target_bir_lowering=False, debug=False)
    zt = nc.dram_tensor("zt", [D, WCOLS], BF16, kind="ExternalInput")
    out1 = nc.dram_tensor("out1", [128, 48], F32, kind="ExternalOutput")

    import ml_dtypes
    negeye_np = (np.eye(128, dtype=np.float32) * np.float32(NEG)).astype(ml_dtypes.bfloat16)
    negeye_d = nc.inline_tensor(negeye_np, name="negeye")
    eye_np = np.eye(128, dtype=np.float32).astype(ml_dtypes.bfloat16)
    eye_d = nc.inline_tensor(eye_np, name="eye_bf")

    with TileContext(nc) as tc, ExitStack() as ctx:
        consts = ctx.enter_context(tc.tile_pool(name="consts", bufs=1))
        big = ctx.enter_context(tc.tile_pool(name="big", bufs=1))
        scrp = ctx.enter_context(tc.tile_pool(name="scrp", bufs=4))
        scrpp = ctx.enter_context(tc.tile_pool(name="scrpp", bufs=2))
        smallp = ctx.enter_context(tc.tile_pool(name="smallp", bufs=2))

        # ---- PSUM: 2x[128,1536] exp slots + [128,512] psC + [128,512] spare
        psm = ctx.enter_context(tc.tile_pool(name="psm", bufs=2, space="PSUM"))
        pscp = ctx.enter_context(tc.tile_pool(name="pscp", bufs=1, space="PSUM"))
        pspare = ctx.enter_context(tc.tile_pool(name="pspare", bufs=1, space="PSUM"))

        # ---- constants / big tensors
        dummy_in = consts.tile([1, 1], F32, name="dummy_in", tag="dummy_in")
        nc.vector.memset(dummy_in[:], 0.0)
        dummy_out = consts.tile([1, 1], F32, name="dummy_out", tag="dummy_out")
        # early Exp: pulls the activation-table load off the critical path
        nc.scalar.activation(out=dummy_out[:], in_=dummy_in[:], func=AFT.Exp)

        zbf = big.tile([128, KT, WCOLS], BF16, name="zbf", tag="zbf")
        znt = big.tile([128, KT, WCOLS], FP8, name="znt", tag="znt")
        invrow = big.tile([1, WCOLS], BF16, name="invrow", tag="invrow")
        n2r0 = big.tile([1, 128], F32, name="n2r0", tag="n2r0")
        d_y = big.tile([1, 128], F32, name="d_y", tag="d_y")
        d_t = big.tile([1, 128], F32, name="d_t", tag="d_t")
        n2sb = big.tile([128, NQ], F32, name="n2sb", tag="n2sb")
        rsq_y = big.tile([128, NQ], F32, name="rsq_y", tag="rsq_y")
        rsq_t = big.tile([128, NQ], F32, name="rsq_t", tag="rsq_t")
        accs = big.tile([128, MT, 5], F32, name="accs", tag="accs")
        nc.vector.memset(accs[:], 0.0)

        negeye = consts.tile([128, 128], BF16, name="negeye", tag="negeye")
        eye_bf = consts.tile([128, 128], BF16, name="eye_bf", tag="eye_bf")
        ones_bf = consts.tile([128, 1], BF16, name="ones_bf", tag="ones_bf")
        nc.vector.memset(ones_bf[:], 1.0)
        ones_f8 = consts.tile([128, 1], FP8, name="ones_f8", tag="ones_f8")
        nc.vector.memset(ones_f8[:], 1.0)
        ones_dr = consts.tile([128, KT, 1], FP8, name="ones_dr", tag="ones_dr")
        nc.vector.memset(ones_dr[:], 1.0)
        ones1 = consts.tile([1, 128], BF16, name="ones1", tag="ones1")
        nc.vector.memset(ones1[:], 1.0)

        # psC: colsum/pos accumulators, one PSUM bank.
        # cols 0..23  : full-block colsums, block k chunk m -> col (k-1)*8+m
        # cols 24..31 : block-4 cell colsums, col-tile v -> col 24+v
        # cols 32..39 : positives (8 chunks of 128)
        # cols 64..103: chain norms;  256..383 (f32): ch0 norm row
        #
        # PSUM start=True lazily zero-arms the whole 2KB bank row, so a bank
        # with many long-lived accumulators must see exactly ONE start (the
        # opener below) and ONE stop (the closer at the end); every other
        # matmul into it uses start=False (first touch of an armed byte
        # writes, later touches accumulate).
        psC = pscp.tile([128, 512], F32, name="psC", tag="psC")
        nc.tensor.matmul(psC[:, 448:449], lhsT=ones1[0:1, :],
                         rhs=ones1[0:1, 0:1], start=True, stop=False)

        def emit_load(ci):
            # all loads on one queue: strict priority order on the shared
            # HWDGE/DMA devices (ch0 first, then consts, then the rest);
            # both k-planes in ONE DMA (3-dim DRAM AP) to halve HWDGE holds
            lo, hi = CHAINS[ci]
            nc.sync.dma_start(
                out=zbf[:, :, lo:hi],
                in_=zt[:, lo:hi].rearrange("(k p) c -> p k c", p=128),
            )
            if ci == 1:
                nc.sync.dma_start(out=negeye[:], in_=negeye_d[:, :])
                nc.sync.dma_start(out=eye_bf[:], in_=eye_d[:, :])

        sq_tiles = {}

        def emit_sq(ci):
            """One Square activation per chain on the (idle-during-boot)
            ScalarE; Square is in the same table set as Exp (no reload)."""
            lo, hi = CHAINS[ci]
            sqt = big.tile([128, KT, hi - lo], BF16, name=f"sq{ci}", tag=f"sq{ci}")
            nc.scalar.square(sqt[:, :, :], zbf[:, :, lo:hi])
            sq_tiles[ci] = sqt

        def emit_norms(ci):
            """Norms via the colsum trick, in PARTITION-OUTER compact
            layout: matmul #q uses lhsT = the stride-nq column comb
            sq[:, k, q::nq], so psum col q holds n2(lo + p*nq + q) on
            partition p.  The uncompact DMA is then the plain
            partition-outer [128,nq] -> [1,w] pattern (baseline-proven)."""
            lo, hi = CHAINS[ci]
            q0, q1 = lo // 128, hi // 128
            nq = q1 - q0
            sqt = sq_tiles[ci]
            sqv = sqt[:, :, :].rearrange("p k (c q) -> p k q c", q=nq)
            for q in range(nq):
                for k in range(KT):
                    nc.tensor.matmul(
                        psC[:, 64 + q0 + q:65 + q0 + q],
                        lhsT=sqv[:, k, q, :],
                        rhs=ones_bf[:, 0:1],
                        start=False, stop=False,
                    )
            nc.vector.tensor_copy(n2sb[:, q0:q1], psC[:, 64 + q0:64 + q1])
            x = n2sb[:, q0:q1]
            xi = n2sb.bitcast(I32)[:, q0:q1]
            y = rsq_y[:, q0:q1]
            yi = rsq_y.bitcast(I32)[:, q0:q1]
            t_ = rsq_t[:, q0:q1]
            invcc = big.tile([128, nq], BF16, name=f"invcc{ci}",
                             tag=f"invcc{ci}")
            _rsqrt_chain(nc, x, y, yi, xi, t_, invcc[:, :])
            # uncompact: invcc[p, q] = inv of col lo + p*nq + q -> invrow
            nc.sync.dma_start(out=invrow[0:1, lo:hi], in_=invcc[:, :])

        def emit_scale(c0, c1, pso_pool, pso_w):
            """znt[:, :, c0:c1] = zbf * inv_col, where inv_col comes from a
            rank-1 PE outer product ones[1,128]^T @ invrow-slice held in
            PSUM and read directly by the DVE scale (no binv tensor)."""
            for g0 in range(c0, c1, pso_w):
                g1 = min(g0 + pso_w, c1)
                pso = pso_pool.tile([128, pso_w], F32, name="pso",
                                    tag="reg" if pso_pool is psm else "spare")
                for s0 in range(g0, g1, 512):
                    s1 = min(s0 + 512, g1)
                    nc.tensor.matmul(
                        pso[:, s0 - g0:s1 - g0], lhsT=ones1[0:1, :],
                        rhs=invrow[0:1, s0:s1], start=True, stop=True,
                    )
                for k in range(KT):
                    nc.vector.tensor_mul(
                        znt[:, k, g0:g1], zbf[:, k, g0:g1], pso[:, 0:g1 - g0])

        def emit_chain0():
            """Cols [0,128): minimal-latency direct path (no DMA hops):
            ones-matmul n2 row + DVE drain + 1-Newton rsqrt on [1,128]
            (rel err ~1e-3; these 128 cols are 2.7% of each row sum)."""
            lo, hi = CHAINS[0]
            emit_sq(0)
            sqt = sq_tiles[0]
            for k in range(KT):
                nc.tensor.matmul(
                    psC[0:1, 256:384], lhsT=ones_bf[:, 0:1], rhs=sqt[:, k, 0:128],
                    start=False, stop=False,
                )
            nc.vector.tensor_copy(n2r0[0:1, :], psC[0:1, 256:384])
            x = n2r0[0:1, :]
            xi = n2r0.bitcast(I32)[0:1, :]
            y = d_y[0:1, :]
            yi = d_y.bitcast(I32)[0:1, :]
            _rsqrt_chain(nc, x, y, yi, xi, d_t[0:1, :], invrow[0:1, lo:hi],
                         newton=1)
            emit_scale(lo, hi, pspare, 512)

        # region list: (m, reg_off, reg_w, acc_t);  m0/r0 split for startup
        regions = []
        for r in range(3):
            for m in range(MT):
                if r == 0 and m == 0:
                    regions.append((0, 0, 128, 0))
                    regions.append((0, 128, 1408, 1))
                else:
                    regions.append((m, 1536 * r, 1536, 2 + r if m == 0 else r))

        # block-4 cell colsum start/stop bookkeeping: psC col 24+v gets
        # contributions from m in sorted({v-1, v-2, v-3} mod 8)
        cell_ms = {v: sorted(((v - k) % 8 for k in range(1, 4))) for v in range(8)}

        pending_colsums = []  # deferred one region for PE pipelining

        def flush_colsums():
            for args in pending_colsums:
                nc.tensor.matmul(**args)
            pending_colsums.clear()

        def emit_region(m, off, w, acc_t):
            reg = psm.tile([128, RW], F32, name="reg", tag="reg")
            # gram chunks (512-aligned within the tile)
            for j in range((w + 511) // 512):
                l0, l1 = 512 * j, min(512 * (j + 1), w)
                g0 = off + l0
                if off == 3072 and l0 == 1024:
                    # r2 cells: 4 x [128,128] at col-tiles (m+k)%8 of block 4
                    for kk in range(4):
                        v = (m + kk) % 8
                        nc.tensor.matmul(
                            reg[:, l0 + 128 * kk:l0 + 128 * (kk + 1)],
                            lhsT=znt[:, :, 128 * m:128 * (m + 1)],
                            rhs=znt[:, :, 4096 + 128 * v:4096 + 128 * (v + 1)],
                            start=True, stop=True,
                            perf_mode=mybir.MatmulPerfMode.DoubleRow,
                        )
                    continue
                has_diag = g0 <= 128 * m and 128 * (m + 1) <= off + l1
                nc.tensor.matmul(
                    reg[:, l0:l1],
                    lhsT=znt[:, :, 128 * m:128 * (m + 1)],
                    rhs=znt[:, :, g0:off + l1],
                    start=True, stop=not has_diag,
                    perf_mode=mybir.MatmulPerfMode.DoubleRow,
                )
                if has_diag:
                    dcol = 128 * m - g0
                    nc.tensor.matmul(
                        reg[:, l0 + dcol:l0 + dcol + 128],
                        lhsT=negeye[:, :], rhs=eye_bf[:, :],
                        start=False, stop=True,
                    )
            flush_colsums()
            scr = scrp.tile([128, RW], FP8, name="scr", tag="scr")
            nc.scalar.activation(
                out=scr[0:128, 0:w], in_=reg[:, 0:w], func=AFT.Exp,
                accum_out=accs[:, m, acc_t:acc_t + 1],
            )
            # colsums for off-diag full-block cols [1024,4096) + r2 cells
            for gc in range(max(off, 1024), min(off + w, 4096), 128):
                lc = gc - off
                pc = (gc - 1024) // 128
                pending_colsums.append(dict(
                    out=psC[:, pc:pc + 1],
                    lhsT=scr[:, lc:lc + 128], rhs=ones_f8[:, 0:1],
                    start=False, stop=False,
                ))
            if off == 3072:
                for kk in range(1, 4):   # cells k=1..3 emit colsums
                    v = (m + kk) % 8
                    lc = 1024 + 128 * kk
                    pc = 24 + v
                    pending_colsums.append(dict(
                        out=psC[:, pc:pc + 1],
                        lhsT=scr[:, lc:lc + 128], rhs=ones_f8[:, 0:1],
                        start=False, stop=False,
                    ))

        # positives: sim(r, 4096+r) via elementwise prod + colsum trick
        prod = big.tile([128, KT, LOCAL], FP8, name="prod", tag="prod")

        def emit_pos_prods():
            for k in range(KT):
                nc.vector.tensor_mul(
                    prod[:, k, :], znt[:, k, 0:LOCAL], znt[:, k, 4096:4096 + LOCAL])

        def emit_pos_mms():
            for t in range(8):
                nc.tensor.matmul(
                    psC[:, 32 + t:33 + t],
                    lhsT=prod[:, :, 128 * t:128 * (t + 1)], rhs=ones_dr[:, :, 0:1],
                    start=False, stop=False,
                    perf_mode=mybir.MatmulPerfMode.DoubleRow,
                )

        # P regions: redundant offset-4 cells (u, u+4), rowsums via
        # segmented reduce (no accum: partitions span 4 different m-tiles)
        p4 = smallp.tile([128, 8], F32, name="p4", tag="p4")

        def emit_p(half):
            preg = pspare.tile([128, 512], F32, name="preg", tag="spare")
            for t in range(4):
                u = 4 * half + t
                v = (u + 4) % 8
                nc.tensor.matmul(
                    preg[:, 128 * t:128 * (t + 1)],
                    lhsT=znt[:, :, 128 * u:128 * (u + 1)],
                    rhs=znt[:, :, 4096 + 128 * v:4096 + 128 * (v + 1)],
                    start=(t == 0), stop=(t == 3),
                    perf_mode=mybir.MatmulPerfMode.DoubleRow,
                )
            scrP = scrpp.tile([128, 512], FP8, name="scrP", tag="scrP")
            nc.scalar.activation(out=scrP[:, 0:512], in_=preg[:, 0:512], func=AFT.Exp)
            nc.vector.reduce_sum(
                out=p4[:, 4 * half:4 * half + 4],
                in_=scrP[:].rearrange("p (c w) -> p c w", w=128),
                axis=mybir.AxisListType.X,
            )

        # ---- emission schedule: chains' latency pieces front-loaded so the
        # exp stream (ScalarE) never starves; squares use ScalarE's idle boot
        # window; pos/P interleave into the stream instead of after it.
        emit_load(0)
        emit_load(1)
        emit_load(2)
        emit_load(3)
        emit_chain0()
        emit_region(*regions[0])        # (m0, [0,128))
        emit_sq(1)
        emit_norms(1)
        emit_sq(2)
        emit_sq(3)
        emit_scale(128, 1536, psm, RW)  # via a borrowed exp slot (1 group)
        emit_region(*regions[1])        # (m0, [128,1536))
        emit_norms(2)
        emit_scale(1536, 3072, pspare, 512)
        emit_region(*regions[2])        # r0 m1
        emit_region(*regions[3])        # r0 m2
        emit_norms(3)
        for i in range(4, 9):           # r0 m3..m7
            emit_region(*regions[i])
        emit_scale(3072, 5120, pspare, 512)
        for i in range(9, 15):          # r1 m0..m5
            emit_region(*regions[i])
        emit_pos_prods()
        for i in range(15, 19):         # r1 m6..m7, r2 m0..m1
            emit_region(*regions[i])
        emit_pos_mms()
        for i in range(19, 24):         # r2 m2..m6
            emit_region(*regions[i])
        emit_p(0)
        emit_region(*regions[24])       # r2 m7
        flush_colsums()
        # P1 last: it emits no colsums, so the psC drain DMA launches
        # while its exp still runs
        emit_p(1)
        # close the psC bank's long accumulation group (sim bookkeeping)
        nc.tensor.matmul(psC[:, 449:450], lhsT=ones1[0:1, :],
                         rhs=ones1[0:1, 0:1], start=False, stop=True)

        # ---- tail: colsum/pos columns finalize ~2 exps before the stream
        # ends, so their DMA overlaps the last exps; only the tiny rowsum
        # DMA trails the final accum.
        sbout = big.tile([128, 48], F32, name="sbout", tag="sbout")
        nc.vector.tensor_copy(sbout[:, 8:48], psC[:, 0:40])
        nc.sync.dma_start(out=out1[:, 8:48], in_=sbout[:, 8:48])
        srow = smallp.tile([128, 8], F32, name="srow", tag="srow")
        nc.vector.reduce_sum(out=srow[:], in_=accs[:], axis=mybir.AxisListType.X)
        nc.vector.tensor_add(sbout[:, 0:8], srow[:], p4[:])
        nc.sync.dma_start(out=out1[:, 0:8], in_=sbout[:, 0:8])

    nc.compile()
    return nc


def build_nc2():
    """Phase 2: S = sum of 5 routed contributions per row; lse = ln S;
    emit per-core [sum_lse, sum_pos]."""
    nc = bacc.Bacc("TRN2", target_bir_lowering=False, debug=False)
    in2 = nc.dram_tensor("in2", [128, 48], F32, kind="ExternalInput")
    out2 = nc.dram_tensor("out2", [2, 1], F32, kind="ExternalOutput")

    with TileContext(nc) as tc, ExitStack() as ctx:
        p = ctx.enter_context(tc.tile_pool(name="p", bufs=1))
        ps = ctx.enter_context(tc.tile_pool(name="ps", bufs=1, space="PSUM"))

        dummy_in = p.tile([1, 1], F32, name="dummy_in", tag="dummy_in")
        nc.vector.memset(dummy_in[:], 1.0)
        dummy_out = p.tile([1, 1], F32, name="dummy_out", tag="dummy_out")
        nc.scalar.activation(out=dummy_out[:], in_=dummy_in[:], func=AFT.Ln)

        vin = p.tile([128, 48], F32, name="vin", tag="vin")
        nc.sync.dma_start(out=vin[:], in_=in2[:, :])

        S = p.tile([128, 8], F32, name="S", tag="S")
        nc.vector.reduce_sum(
            out=S[:],
            in_=vin[:, 0:40].rearrange("p (m j) -> p m j", j=5),
            axis=mybir.AxisListType.X,
        )
        lse = p.tile([128, 8], F32, name="lse", tag="lse")
        nc.scalar.activation(out=lse[:], in_=S[:], func=AFT.Ln)

        pk = p.tile([128, 2], F32, name="pk", tag="pk")
        nc.vector.reduce_sum(out=pk[:, 0:1], in_=lse[:], axis=mybir.AxisListType.X)
        nc.vector.reduce_sum(out=pk[:, 1:2], in_=vin[:, 40:48], axis=mybir.AxisListType.X)

        ones_f32 = p.tile([128, 1], F32, name="ones_f32", tag="ones_f32")
        nc.vector.memset(ones_f32[:], 1.0)
        tot = ps.tile([128, 512], F32, name="tot", tag="tot")
        nc.tensor.matmul(tot[0:2, 0:1], lhsT=pk[:, 0:2], rhs=ones_f32[:, 0:1],
                         start=True, stop=True)
        osb = p.tile([2, 1], F32, name="osb", tag="osb")
        nc.vector.tensor_copy(osb[:], tot[0:2, 0:1])
        nc.sync.dma_start(out=out2[:, :], in_=osb[:])

    nc.compile()
    return nc


_NC_CACHE = [None, None]


def _get_ncs():
    if _NC_CACHE[0] is None:
        _NC_CACHE[0] = build_nc1()
        _NC_CACHE[1] = build_nc2()
    return _NC_CACHE


def make_in_maps(z1: np.ndarray, z2: np.ndarray):
    import ml_dtypes
    z = np.concatenate([np.asarray(z1, np.float32), np.asarray(z2, np.float32)], axis=0)
    zT = np.ascontiguousarray(z.T)                      # (256, 8192) f32
    zTb = zT.astype(ml_dtypes.bfloat16)
    in_maps = []
    for c in range(NCORES):
        rolled = np.roll(zTb, -c * LOCAL, axis=1)[:, :WCOLS]
        in_maps.append({"zt": np.ascontiguousarray(rolled)})
    return in_maps


def route_phase2(outs1):
    """outs1: list of 8 [128,48] f32 arrays -> list of 8 [128,48] phase-2
    inputs.  Pure permutation/gather of columns (no arithmetic).

    out1 cols: 0..7  partial rowsums (m);  8+q colsum psC col q
               (q=(k-1)*8+m for full blocks k=1..3; q=24+v for block-4
               cells; q=32+t positives);  col j of in2: 5m+j layout."""
    in2 = []
    for s in range(NCORES):
        arr = np.zeros((128, 48), np.float32)
        own = outs1[s]
        for m in range(8):
            arr[:, 5 * m + 0] = own[:, m]
            for j in (1, 2, 3):
                peer = outs1[(s - j) % NCORES]
                arr[:, 5 * m + j] = peer[:, 8 + (j - 1) * 8 + m]
            peer4 = outs1[(s + 4) % NCORES]
            arr[:, 5 * m + 4] = peer4[:, 8 + 24 + m]
        arr[:, 40:48] = own[:, 40:48]
        in2.append({"in2": arr})
    return in2


def combine(parts2):
    """parts2: 8 x [2,1] arrays [sum_lse; sum_pos] -> scalar loss."""
    sum_lse = sum(float(p[0, 0]) for p in parts2)
    sum_pos = sum(float(p[1, 0]) for p in parts2[NCORES // 2:])
    loss = (sum_lse + float(B) * 1.0e9 - sum_pos) / float(NTOT)
    return np.float32(loss)


def kernel(z1: np.ndarray, z2: np.ndarray) -> np.ndarray:
    nc1, nc2 = _get_ncs()
    in_maps = make_in_maps(z1, z2)
    res1 = run_bass_kernel_spmd(nc1, in_maps, core_ids=list(range(NCORES)))
    outs1 = [np.asarray(r["out1"], np.float32) for r in res1.results]
    in2 = route_phase2(outs1)
    res2 = run_bass_kernel_spmd(nc2, in2, core_ids=list(range(NCORES)))
    parts2 = [np.asarray(r["out2"], np.float32) for r in res2.results]
    return combine(parts2)
